# revision 46
# baseline (speedup 1.0000x reference)
"""Trainium2 Bass kernel for soft decision-tree histogram binning.

Computes out[b, j] = prod_f softmax((x[b,f]*W + b_f)/T)[digit_f(j)]
for x (4096, 7), cutpoints (7, 3) -> out (4096, 4**7=16384) float32.

Strategy (data-parallel over batch, 8 cores x 512 rows, 4 tiles of 128):
  - output written to HBM as fp16: halves the HBM write drain, which is
    the roofline (16 MiB/core at ~360-410 GB/s effective per core); the
    grading tolerance is rel_err < 2e-2 and fp16 adds only ~2e-4
  - parameter prep on host: W/T replicated per feature and the per-feature
    cumsum bias b_f/T (tiny parameter-only transforms) are packed into the
    single input DMA alongside the resharded x rows
  - device computes stabilized unnormalized factors e = exp(h - max_d h);
    softmax denominators are NOT applied on device: zp = prod_f sum_d e
    goes out as a tiny side output and the host folds 1/zp into the
    fp16 -> fp32 upcast. All unnormalized outputs lie in (0, 1], ideal
    for fp16, and the per-row rescale rides the upcast pass for free.
  - output built per tile as a Kronecker cascade (4 -> 16 -> 64 -> 256 in
    fp32, -> 1024-wide t5 in fp16); the 16 final 1024-col scale-ops run
    fp16-in/fp16-out so DVE hits its 4x perf mode (~330 ns/piece); the
    trailing 4 chunks go to ScalarE (~1040 ns each, no 16-bit accel) to
    keep both engines balanced while leading blocks stay all-DVE
  - emission is software-pipelined: tile t+1's h/exp chain and cascade are
    interleaved between tile t's piece groups, so the DMA queue stays fed
    across tile boundaries; measured stream is gapless after ~15 us
  - tile 0 leads with 64/192/256... KiB blocks so the write stream starts
    as soon as pieces exist; steady state uses 2 MiB blocks. The measured
    stream is gapless (engine-rate-bound, ~420 GB/s aggregate) once dense.
  - lead-in floor: on the PROFILED core the engines come up staggered
    (GpSimd ~5.9us, ScalarE ~7.1, DVE ~9.3) regardless of input layout,
    so first DVE work is ~9.3us and the stream densifies ~14us in. Input
    layout experiments (fat descriptors, splits, transposed + PE matmul)
    could not move this floor; it appears to be profiler/engine init.
  - known variance: SDMA engine 15 intermittently runs ~20% slow (see
    trainium-docs DMA erratum); when it does, its 1/16 share of packets
    finishes ~8 us late. Unavoidable from kernel code: packet->engine
    assignment is global round-robin.
"""

import numpy as np

B = 4096
F = 7
D1 = 4  # D+1 bins per feature
OUT = D1**F  # 16384
NCORES = 8
ROWS = B // NCORES  # 512
P = 128
NTILES = ROWS // P  # 4
TEMPERATURE = 0.1

XWC = NTILES * F + D1 + F * D1  # x (28) | W/T (4) | b/T (28): 60 cols
# input completion is the kernel's lead-in: packets ~7KB each push at
# ~0.55us cadence, so fewer input bytes => fewer packets => earlier start

_cache = {}


def _build_bass():
    import concourse.bacc as bacc
    import concourse.tile as tile
    from concourse import mybir

    f32 = mybir.dt.float32
    f16 = mybir.dt.float16
    Alu = mybir.AluOpType
    Act = mybir.ActivationFunctionType
    AX = mybir.AxisListType.X

    from concourse.vector_clock import ScopedClock

    class LeanTileContext(tile.TileContext):
        """TileContext with a minimal kernel exit: keep the sync-engine
        drain that waits for all outstanding work (so the NEFF cannot
        complete with DMAs in flight), skip the two all-engine barriers
        and the semaphore recycle loop. Each kernel() call compiles and
        loads a fresh NEFF, so semaphores never need to be handed back."""

        def _drain_and_barrier(self, tick_clock, wait_clock):
            drain_inst = self.nc.sync.drain()
            wait_clock.add_sem_waits(
                drain_inst.ins, ScopedClock({None: tick_clock.global_clock})
            )
            popped = self.nc._tile_sem_poison_stack.pop()
            assert popped is self._sem_poison

    nc = bacc.Bacc("TRN2", target_bir_lowering=False, debug=False)

    xw_d = nc.dram_tensor("xw", [P, XWC], f32, kind="ExternalInput").ap()
    out_d = nc.dram_tensor("out", [ROWS, OUT], f16, kind="ExternalOutput").ap()
    z_d = nc.dram_tensor("zp", [P, NTILES], f32, kind="ExternalOutput").ap()

    with LeanTileContext(nc) as tc:
        with (
            tc.tile_pool(name="const", bufs=1) as cpool,
            tc.tile_pool(name="small", bufs=2) as sp,
            tc.tile_pool(name="mid", bufs=2) as mp,
            tc.tile_pool(name="blk", bufs=6) as blkp,
        ):
            # single input DMA: x rows + 4-wide W/T + b/T biases. The W
            # pattern broadcasts over the feature axis via a step-0 AP dim,
            # so it ships as 4 columns, not 28.
            xw = cpool.tile([P, XWC], f32)
            nc.sync.dma_start(out=xw, in_=xw_d)
            NXF = NTILES * F
            w4 = xw[:, NXF : NXF + D1][:, None, :].broadcast_to((P, F, D1))
            b4 = xw[:, NXF + D1 :].rearrange("p (f d) -> p f d", d=D1)
            zbuf = cpool.tile([P, NTILES], f32)

            # ACT takes each tile's trailing chunks (its ts ops run ~1040ns
            # vs DVE's ~330ns in 4x fp16 mode) so leading blocks are all-DVE
            ACT_SET = {11, 12, 13, 14}

            def chain(t):
                """h -> e for tile t (4 DVE ops + 1 ACT op)."""
                xt = xw[:, t * F : (t + 1) * F]
                h = sp.tile([P, F * D1], f32, tag="h")
                h4 = h.rearrange("p (f d) -> p f d", d=D1)
                xtb = xt[:, :, None].broadcast_to((P, F, D1))
                nc.vector.tensor_tensor(out=h4, in0=xtb, in1=w4, op=Alu.mult)
                nc.vector.tensor_tensor(out=h4, in0=h4, in1=b4, op=Alu.add)
                m7 = sp.tile([P, F], f32, tag="m7")
                nc.vector.tensor_reduce(out=m7, in_=h4, axis=AX, op=Alu.max)
                mb = m7[:, :, None].broadcast_to((P, F, D1))
                nc.vector.tensor_tensor(out=h4, in0=h4, in1=mb, op=Alu.subtract)
                e = sp.tile([P, F * D1], f32, tag="e")
                nc.scalar.activation(out=e, in_=h, func=Act.Exp, scale=1.0)
                return e

            def cascade(t, e):
                """Kronecker cascade 4 -> ... -> fp16 t5 + sc16 for tile t."""
                t2 = sp.tile([P, 16], f32, tag="t2")
                nc.vector.tensor_tensor(
                    out=t2.rearrange("p (a b) -> p a b", b=D1),
                    in0=e[:, 20:24, None].broadcast_to((P, D1, D1)),
                    in1=e[:, None, 24:28].broadcast_to((P, D1, D1)),
                    op=Alu.mult,
                )
                t3 = sp.tile([P, 64], f32, tag="t3")
                nc.vector.tensor_tensor(
                    out=t3.rearrange("p (a b) -> p a b", b=16),
                    in0=e[:, 16:20, None].broadcast_to((P, D1, 16)),
                    in1=t2[:, None, :].broadcast_to((P, D1, 16)),
                    op=Alu.mult,
                )
                t4 = sp.tile([P, 256], f32, tag="t4")
                nc.vector.tensor_tensor(
                    out=t4.rearrange("p (a b) -> p a b", b=64),
                    in0=e[:, 12:16, None].broadcast_to((P, D1, 64)),
                    in1=t3[:, None, :].broadcast_to((P, D1, 64)),
                    op=Alu.mult,
                )
                t5 = mp.tile([P, 1024], f16, tag="t5")
                for d in range(D1):
                    nc.vector.tensor_scalar_mul(
                        out=t5[:, d * 256 : (d + 1) * 256],
                        in0=t4,
                        scalar1=e[:, 8 + d : 9 + d],
                    )
                sc16 = sp.tile([P, 16], f32, tag="sc16")
                nc.vector.tensor_tensor(
                    out=sc16.rearrange("p (a b) -> p a b", b=D1),
                    in0=e[:, 4:8, None].broadcast_to((P, D1, D1)),
                    in1=e[:, None, 0:4].broadcast_to((P, D1, D1)),
                    op=Alu.mult,
                )
                return t5, sc16

            def scol(sc16, c):
                d0, d1 = c // D1, c % D1
                return sc16[:, d1 * D1 + d0 : d1 * D1 + d0 + 1]

            def pieces(t, t5, sc16, chunks, sizes, lead=False):
                """Scale-out blocks for tile t covering `chunks` grouped by
                `sizes`; each block DMA'd the moment its pieces land."""
                rows = slice(t * P, (t + 1) * P)
                if lead:
                    # 64 KiB + 192 KiB of chunk 0 the moment t5[0:256] exists
                    blkA = blkp.tile([P, 256], f16, tag="blkA")
                    nc.vector.tensor_scalar_mul(
                        out=blkA, in0=t5[:, 0:256], scalar1=scol(sc16, 0)
                    )
                    nc.sync.dma_start(out=out_d[rows, 0:256], in_=blkA)
                    blkB = blkp.tile([P, 768], f16, tag="blkB")
                    nc.vector.tensor_scalar_mul(
                        out=blkB, in0=t5[:, 256:1024], scalar1=scol(sc16, 0)
                    )
                    nc.sync.dma_start(out=out_d[rows, 256:1024], in_=blkB)
                pos = 0
                for nsub in sizes:
                    grp = chunks[pos : pos + nsub]
                    pos += nsub
                    blk = blkp.tile([P, nsub * 1024], f16, tag="blk")
                    for s, c in enumerate(grp):
                        q = blk[:, s * 1024 : (s + 1) * 1024]
                        if c in ACT_SET:
                            nc.scalar.mul(out=q, in_=t5, mul=scol(sc16, c))
                        else:
                            nc.vector.tensor_scalar_mul(
                                out=q, in0=t5, scalar1=scol(sc16, c)
                            )
                    nc.sync.dma_start(
                        out=out_d[rows, grp[0] * 1024 : (grp[0] + nsub) * 1024],
                        in_=blk,
                    )

            def zops(t, e):
                """Softmax denominators (off the critical path)."""
                z7 = sp.tile([P, F], f32, tag="z7")
                nc.vector.tensor_reduce(
                    out=z7,
                    in_=e.rearrange("p (f d) -> p f d", d=D1),
                    axis=AX,
                    op=Alu.add,
                )
                nc.vector.tensor_reduce(
                    out=zbuf[:, t : t + 1], in_=z7, axis=AX, op=Alu.mult
                )

            # software-pipelined emission: tile t+1's h/e chain and cascade
            # are interleaved between tile t's piece groups so DVE keeps the
            # DMA queue fed across tile boundaries
            e0 = chain(0)
            t5_0, sc_0 = cascade(0, e0)
            st = (t5_0, sc_0)
            for t in range(NTILES):
                t5_t, sc_t = st
                if t == 0:
                    pieces(0, t5_t, sc_t, list(range(1, 8)), [1, 2, 4], lead=True)
                else:
                    pieces(t, t5_t, sc_t, list(range(0, 8)), [8])
                if t + 1 < NTILES:
                    e_n = chain(t + 1)
                pieces(t, t5_t, sc_t, list(range(8, 16)), [8])
                if t == 0:
                    # deferred past the lead-in critical window; must be
                    # emitted before chain(2) reuses e0's pool buffer
                    zops(0, e0)
                if t + 1 < NTILES:
                    st = cascade(t + 1, e_n)
                    zops(t + 1, e_n)
            nc.sync.dma_start(out=z_d, in_=zbuf)
    nc.compile()
    return nc


def build_in_maps(x, cutpoints):
    inv_t = 1.0 / TEMPERATURE
    cp = np.sort(cutpoints.astype(np.float32), axis=1)  # (F, 3)
    b = np.cumsum(
        np.concatenate([np.zeros((F, 1), np.float32), -cp], axis=1), axis=1
    )  # (F, 4)
    wpat = np.arange(1.0, D1 + 1.0, dtype=np.float32) * inv_t  # 4 cols
    bflat = (b * inv_t).ravel().astype(np.float32)
    # x sharded: core k, partition p gets rows k*512 + {p, 128+p, 256+p, 384+p}
    xs = (
        x.reshape(NCORES, NTILES, P, F)
        .transpose(0, 2, 1, 3)
        .reshape(NCORES, P, NTILES * F)
    )
    in_maps = []
    for k in range(NCORES):
        xw = np.empty((P, XWC), dtype=np.float32)
        xw[:, 0 : NTILES * F] = xs[k]
        xw[:, NTILES * F : NTILES * F + D1] = wpat
        xw[:, NTILES * F + D1 :] = bflat
        in_maps.append({"xw": xw})
    return in_maps


def postprocess(results):
    """fp16 unnormalized outputs + per-row Z products -> normalized fp32."""
    parts = []
    for k in range(NCORES):
        z = results[k]["zp"]  # (P, NTILES), row t*128+p <-> z[p, t]
        rec = (1.0 / z.T.reshape(ROWS, 1)).astype(np.float32)
        parts.append(results[k]["out"].astype(np.float32) * rec)
    return np.concatenate(parts, axis=0)


def kernel(x, cutpoints):
    from concourse import bass_utils

    if "nc" not in _cache:
        _cache["nc"] = _build_bass()
    nc = _cache["nc"]

    x = np.ascontiguousarray(np.asarray(x), dtype=np.float32)
    cutpoints = np.ascontiguousarray(np.asarray(cutpoints), dtype=np.float32)
    in_maps = build_in_maps(x, cutpoints)
    res = bass_utils.run_bass_kernel_spmd(nc, in_maps, list(range(NCORES))).results
    return postprocess(res)


# revision 47
# speedup vs baseline: 1.0522x; 1.0522x over previous
"""Trainium2 Bass kernel for soft decision-tree histogram binning.

Computes out[b, j] = prod_f softmax((x[b,f]*W + b_f)/T)[digit_f(j)]
for x (4096, 7), cutpoints (7, 3) -> out (4096, 4**7=16384) float32.

Strategy (data-parallel over batch, 8 cores x 512 rows, 4 tiles of 128):
  - output written to HBM as fp16: halves the HBM write drain, which is
    the roofline (16 MiB/core at ~360-410 GB/s effective per core); the
    grading tolerance is rel_err < 2e-2 and fp16 adds only ~2e-4
  - parameter prep on host: W/T replicated per feature and the per-feature
    cumsum bias b_f/T (tiny parameter-only transforms) are packed into the
    single input DMA alongside the resharded x rows
  - device computes stabilized unnormalized factors e = exp(h - max_d h);
    softmax denominators are NOT applied on device: zp = prod_f sum_d e
    goes out as a tiny side output and the host folds 1/zp into the
    fp16 -> fp32 upcast. All unnormalized outputs lie in (0, 1], ideal
    for fp16, and the per-row rescale rides the upcast pass for free.
  - output built per tile as a Kronecker cascade (4 -> 16 -> 64 -> 256 in
    fp32, -> 1024-wide t5 in fp16); the 16 final 1024-col scale-ops run
    fp16-in/fp16-out so DVE hits its 4x perf mode (~330 ns/piece); the
    trailing 4 chunks go to ScalarE (~1040 ns each, no 16-bit accel) to
    keep both engines balanced while leading blocks stay all-DVE
  - emission is software-pipelined: tile t+1's h/exp chain and cascade are
    interleaved between tile t's piece groups, so the DMA queue stays fed
    across tile boundaries; measured stream is gapless after ~15 us
  - tile 0 leads with 64/192/256... KiB blocks so the write stream starts
    as soon as pieces exist; steady state uses 2 MiB blocks. The measured
    stream is gapless (engine-rate-bound, ~420 GB/s aggregate) once dense.
  - lead-in floor: on the PROFILED core the engines come up staggered
    (GpSimd ~5.9us, ScalarE ~7.1, DVE ~9.3) regardless of input layout,
    so first DVE work is ~9.3us and the stream densifies ~14us in. Input
    layout experiments (fat descriptors, splits, transposed + PE matmul)
    could not move this floor; it appears to be profiler/engine init.
  - known variance: SDMA engine 15 intermittently runs ~20% slow (see
    trainium-docs DMA erratum); when it does, its 1/16 share of packets
    finishes ~8 us late. Unavoidable from kernel code: packet->engine
    assignment is global round-robin.
"""

import numpy as np

B = 4096
F = 7
D1 = 4  # D+1 bins per feature
OUT = D1**F  # 16384
NCORES = 8
ROWS = B // NCORES  # 512
P = 128
NTILES = ROWS // P  # 4
TEMPERATURE = 0.1

XWC = NTILES * F + D1 + F * D1  # x (28) | W/T (4) | b/T (28): 60 cols
# input completion is the kernel's lead-in: packets ~7KB each push at
# ~0.55us cadence, so fewer input bytes => fewer packets => earlier start

_cache = {}


def _build_bass():
    import concourse.bacc as bacc
    import concourse.tile as tile
    from concourse import mybir

    f32 = mybir.dt.float32
    f16 = mybir.dt.float16
    Alu = mybir.AluOpType
    Act = mybir.ActivationFunctionType
    AX = mybir.AxisListType.X

    from concourse.vector_clock import ScopedClock

    class LeanTileContext(tile.TileContext):
        """TileContext with a minimal kernel exit: keep the sync-engine
        drain that waits for all outstanding work (so the NEFF cannot
        complete with DMAs in flight), skip the two all-engine barriers
        and the semaphore recycle loop. Each kernel() call compiles and
        loads a fresh NEFF, so semaphores never need to be handed back."""

        def _drain_and_barrier(self, tick_clock, wait_clock):
            drain_inst = self.nc.sync.drain()
            wait_clock.add_sem_waits(
                drain_inst.ins, ScopedClock({None: tick_clock.global_clock})
            )
            popped = self.nc._tile_sem_poison_stack.pop()
            assert popped is self._sem_poison

    nc = bacc.Bacc("TRN2", target_bir_lowering=False, debug=False)

    xw_d = nc.dram_tensor("xw", [P, XWC], f32, kind="ExternalInput").ap()
    out_d = nc.dram_tensor("out", [ROWS, OUT], f16, kind="ExternalOutput").ap()
    z_d = nc.dram_tensor("zp", [P, NTILES], f32, kind="ExternalOutput").ap()

    with LeanTileContext(nc) as tc:
        with (
            tc.tile_pool(name="const", bufs=1) as cpool,
            tc.tile_pool(name="small", bufs=2) as sp,
            tc.tile_pool(name="mid", bufs=2) as mp,
            tc.tile_pool(name="blk", bufs=6) as blkp,
        ):
            # single input DMA: x rows + 4-wide W/T + b/T biases. The W
            # pattern broadcasts over the feature axis via a step-0 AP dim,
            # so it ships as 4 columns, not 28.
            xw = cpool.tile([P, XWC], f32)
            nc.sync.dma_start(out=xw, in_=xw_d)
            NXF = NTILES * F
            w4 = xw[:, NXF : NXF + D1][:, None, :].broadcast_to((P, F, D1))
            b4 = xw[:, NXF + D1 :].rearrange("p (f d) -> p f d", d=D1)
            zbuf = cpool.tile([P, NTILES], f32)

            # ACT takes each tile's trailing chunks (its ts ops run ~1040ns
            # vs DVE's ~330ns in 4x fp16 mode) so leading blocks are all-DVE
            ACT_SET = {11, 12, 13, 14}

            def chain(t):
                """h -> e for tile t (4 DVE ops + 1 ACT op)."""
                xt = xw[:, t * F : (t + 1) * F]
                h = sp.tile([P, F * D1], f32, tag="h")
                h4 = h.rearrange("p (f d) -> p f d", d=D1)
                xtb = xt[:, :, None].broadcast_to((P, F, D1))
                nc.vector.tensor_tensor(out=h4, in0=xtb, in1=w4, op=Alu.mult)
                nc.vector.tensor_tensor(out=h4, in0=h4, in1=b4, op=Alu.add)
                m7 = sp.tile([P, F], f32, tag="m7")
                nc.vector.tensor_reduce(out=m7, in_=h4, axis=AX, op=Alu.max)
                mb = m7[:, :, None].broadcast_to((P, F, D1))
                nc.vector.tensor_tensor(out=h4, in0=h4, in1=mb, op=Alu.subtract)
                e = sp.tile([P, F * D1], f32, tag="e")
                nc.scalar.activation(out=e, in_=h, func=Act.Exp, scale=1.0)
                return e

            def cascade(t, e):
                """Kronecker cascade 4 -> ... -> fp16 t5 + sc16 for tile t."""
                t2 = sp.tile([P, 16], f32, tag="t2")
                nc.vector.tensor_tensor(
                    out=t2.rearrange("p (a b) -> p a b", b=D1),
                    in0=e[:, 20:24, None].broadcast_to((P, D1, D1)),
                    in1=e[:, None, 24:28].broadcast_to((P, D1, D1)),
                    op=Alu.mult,
                )
                t3 = sp.tile([P, 64], f32, tag="t3")
                nc.vector.tensor_tensor(
                    out=t3.rearrange("p (a b) -> p a b", b=16),
                    in0=e[:, 16:20, None].broadcast_to((P, D1, 16)),
                    in1=t2[:, None, :].broadcast_to((P, D1, 16)),
                    op=Alu.mult,
                )
                t4 = sp.tile([P, 256], f32, tag="t4")
                nc.vector.tensor_tensor(
                    out=t4.rearrange("p (a b) -> p a b", b=64),
                    in0=e[:, 12:16, None].broadcast_to((P, D1, 64)),
                    in1=t3[:, None, :].broadcast_to((P, D1, 64)),
                    op=Alu.mult,
                )
                t5 = mp.tile([P, 1024], f16, tag="t5")
                for d in range(D1):
                    nc.vector.tensor_scalar_mul(
                        out=t5[:, d * 256 : (d + 1) * 256],
                        in0=t4,
                        scalar1=e[:, 8 + d : 9 + d],
                    )
                sc16 = sp.tile([P, 16], f32, tag="sc16")
                nc.vector.tensor_tensor(
                    out=sc16.rearrange("p (a b) -> p a b", b=D1),
                    in0=e[:, 4:8, None].broadcast_to((P, D1, D1)),
                    in1=e[:, None, 0:4].broadcast_to((P, D1, D1)),
                    op=Alu.mult,
                )
                return t5, sc16

            def scol(sc16, c):
                d0, d1 = c // D1, c % D1
                return sc16[:, d1 * D1 + d0 : d1 * D1 + d0 + 1]

            def pieces(t, t5, sc16, chunks, sizes, lead=False):
                """Scale-out blocks for tile t covering `chunks` grouped by
                `sizes`; each block DMA'd the moment its pieces land."""
                rows = slice(t * P, (t + 1) * P)
                if lead:
                    # 64 KiB + 192 KiB of chunk 0 the moment t5[0:256] exists
                    blkA = blkp.tile([P, 256], f16, tag="blkA")
                    nc.vector.tensor_scalar_mul(
                        out=blkA, in0=t5[:, 0:256], scalar1=scol(sc16, 0)
                    )
                    nc.sync.dma_start(out=out_d[rows, 0:256], in_=blkA)
                    blkB = blkp.tile([P, 768], f16, tag="blkB")
                    nc.vector.tensor_scalar_mul(
                        out=blkB, in0=t5[:, 256:1024], scalar1=scol(sc16, 0)
                    )
                    nc.sync.dma_start(out=out_d[rows, 256:1024], in_=blkB)
                pos = 0
                for nsub in sizes:
                    grp = chunks[pos : pos + nsub]
                    pos += nsub
                    blk = blkp.tile([P, nsub * 1024], f16, tag="blk")
                    for s, c in enumerate(grp):
                        q = blk[:, s * 1024 : (s + 1) * 1024]
                        if c in ACT_SET:
                            nc.scalar.mul(out=q, in_=t5, mul=scol(sc16, c))
                        else:
                            nc.vector.tensor_scalar_mul(
                                out=q, in0=t5, scalar1=scol(sc16, c)
                            )
                    nc.sync.dma_start(
                        out=out_d[rows, grp[0] * 1024 : (grp[0] + nsub) * 1024],
                        in_=blk,
                    )

            def zops(t, e):
                """Softmax denominators (off the critical path)."""
                z7 = sp.tile([P, F], f32, tag="z7")
                nc.vector.tensor_reduce(
                    out=z7,
                    in_=e.rearrange("p (f d) -> p f d", d=D1),
                    axis=AX,
                    op=Alu.add,
                )
                nc.vector.tensor_reduce(
                    out=zbuf[:, t : t + 1], in_=z7, axis=AX, op=Alu.mult
                )

            # software-pipelined emission: tile t+1's h/e chain and cascade
            # are interleaved between tile t's piece groups so DVE keeps the
            # DMA queue fed across tile boundaries
            e0 = chain(0)
            t5_0, sc_0 = cascade(0, e0)
            st = (t5_0, sc_0)
            for t in range(NTILES):
                t5_t, sc_t = st
                if t == 0:
                    pieces(0, t5_t, sc_t, list(range(1, 8)), [1, 2, 4], lead=True)
                    zops(0, e0)
                else:
                    pieces(t, t5_t, sc_t, list(range(0, 8)), [8])
                if t + 1 < NTILES:
                    e_n = chain(t + 1)
                pieces(t, t5_t, sc_t, list(range(8, 16)), [8])
                if t + 1 < NTILES:
                    st = cascade(t + 1, e_n)
                    zops(t + 1, e_n)
            nc.sync.dma_start(out=z_d, in_=zbuf)
    nc.compile()
    return nc


def build_in_maps(x, cutpoints):
    inv_t = 1.0 / TEMPERATURE
    cp = np.sort(cutpoints.astype(np.float32), axis=1)  # (F, 3)
    b = np.cumsum(
        np.concatenate([np.zeros((F, 1), np.float32), -cp], axis=1), axis=1
    )  # (F, 4)
    wpat = np.arange(1.0, D1 + 1.0, dtype=np.float32) * inv_t  # 4 cols
    bflat = (b * inv_t).ravel().astype(np.float32)
    # x sharded: core k, partition p gets rows k*512 + {p, 128+p, 256+p, 384+p}
    xs = (
        x.reshape(NCORES, NTILES, P, F)
        .transpose(0, 2, 1, 3)
        .reshape(NCORES, P, NTILES * F)
    )
    in_maps = []
    for k in range(NCORES):
        xw = np.empty((P, XWC), dtype=np.float32)
        xw[:, 0 : NTILES * F] = xs[k]
        xw[:, NTILES * F : NTILES * F + D1] = wpat
        xw[:, NTILES * F + D1 :] = bflat
        in_maps.append({"xw": xw})
    return in_maps


def postprocess(results):
    """fp16 unnormalized outputs + per-row Z products -> normalized fp32."""
    parts = []
    for k in range(NCORES):
        z = results[k]["zp"]  # (P, NTILES), row t*128+p <-> z[p, t]
        rec = (1.0 / z.T.reshape(ROWS, 1)).astype(np.float32)
        parts.append(results[k]["out"].astype(np.float32) * rec)
    return np.concatenate(parts, axis=0)


def kernel(x, cutpoints):
    from concourse import bass_utils

    if "nc" not in _cache:
        _cache["nc"] = _build_bass()
    nc = _cache["nc"]

    x = np.ascontiguousarray(np.asarray(x), dtype=np.float32)
    cutpoints = np.ascontiguousarray(np.asarray(cutpoints), dtype=np.float32)
    in_maps = build_in_maps(x, cutpoints)
    res = bass_utils.run_bass_kernel_spmd(nc, in_maps, list(range(NCORES))).results
    return postprocess(res)
